# revision 48
# baseline (speedup 1.0000x reference)
"""Trainium2 Bass kernel for nn_AttentionBlock (B=4, C=256, H=W=64, 4 heads,
GroupNorm(16) + qkv 1x1 + attention + proj 1x1 + residual).

Sharding: 16 (batch, head) units across 8 cores -> 2 heads (same batch) per
core. Each core computes GroupNorm + qkv for its batch (replicated across the
2 cores sharing a batch), attention for its 2 heads, and a partial proj over
its 128 output-side channels. Host sums the two partials per batch and adds
the residual x plus all constant biases (proj bias + proj(v-bias)).

Key device-side structure:
- GroupNorm is folded into the qkv weights (w' = w*scale, bias' = w^T bi + b)
  so the qkv/vT matmuls consume raw x (cast once to bf16) with no separate
  normalization pass.
- Per s-tile j the two heads' QK matmuls are K=64 in disjoint PE row groups
  (partitions 0-63 / 64-127) and run concurrently.
- exp is split across engines per j: ACT computes true exp, DVE computes a
  Schraudolph-style bit-trick exp (f32 -> int16 affine, bitcast to bf16;
  ~3.3% per-element, washes out through softmax normalization).
- proj + output DMA are folded into the attention t-chunk loop.
"""
import os
import numpy as np
import ml_dtypes
from contextlib import ExitStack

import concourse.bass as bass
import concourse.bacc as bacc
import concourse.tile as tile
from concourse import mybir
from concourse.bass_utils import run_bass_kernel_spmd

F32 = mybir.dt.float32
F32R = mybir.dt.float32r
BF16 = mybir.dt.bfloat16
I16 = mybir.dt.int16
I8 = mybir.dt.int8
F8E4 = mybir.dt.float8e4
F8E5 = mybir.dt.float8e5

B, C, HH, WW = 4, 256, 64, 64
T = HH * WW          # 4096
NHEAD = 4            # heads per batch (2 per core)
CH = 64              # channels per head
EPS = 1e-5
SCALE2 = 0.125       # 1/sqrt(ch) applied inside exp
N_CORES = 8
TC = 512             # attention t-chunk (1 PSUM bank)
NST = T // 128       # 32 s-tiles
NTC = T // TC        # 8 t-chunks
# Schraudolph-style exp for the DVE path: bf16 bits of exp(SCALE2*x) are
# approximated by round(x*SCH_A + SCH_B) computed as f32 -> int16 convert,
# then the int16 tile is bitcast to bf16. Max per-element rel err ~3.3%,
# which washes out through the softmax normalization (verified 7.6e-4
# end-to-end with ALL tiles approximated).
SCH_A = 23.083120654223414   # SCALE2 * log2(e) * 128
SCH_B = 16250.4
# e5m2 variant for the fp8 AV path: int8 bits of fp8e5m2(exp(SCALE2*x))
SCH_A8 = 0.7213475204444817  # SCALE2 * log2(e) * 4
SCH_B8 = 59.8
NJP = NST // 2               # 16 s-tile pairs (DoubleRow AV contracts 2 tiles)
ACT_J = 17           # of every 32 j s-tiles, this many exp on ACT, rest DVE


def _emit(tc_ctx):
    nc = tc_ctx.nc
    tc = tc_ctx

    d_xb = nc.dram_tensor("xb", [2, 128, T], F32, kind="ExternalInput").ap()
    d_wqk = nc.dram_tensor("wqk", [2, 128, 256], BF16, kind="ExternalInput").ap()
    d_bqk = nc.dram_tensor("bqk", [128, 2], F32, kind="ExternalInput").ap()
    d_wv = nc.dram_tensor("wv", [2, 128, 128], BF16, kind="ExternalInput").ap()
    d_gnw = nc.dram_tensor("gnw", [128, 2], F32, kind="ExternalInput").ap()
    d_gnb = nc.dram_tensor("gnb", [128, 2], F32, kind="ExternalInput").ap()
    d_gagg = nc.dram_tensor("gagg", [128, 128], F32R, kind="ExternalInput").ap()
    d_pw = nc.dram_tensor("pw", [2, 64, 256], F32R, kind="ExternalInput").ap()
    d_cones8 = nc.dram_tensor("cones8", [128, 64], F8E4, kind="ExternalInput").ap()
    d_out = nc.dram_tensor("out", [2, 128, T], F32, kind="ExternalOutput").ap()

    with ExitStack() as ctx:
        persist = ctx.enter_context(tc.tile_pool(name="persist", bufs=1))
        small = ctx.enter_context(tc.tile_pool(name="small", bufs=1))

        # ---- persistent SBUF ----
        xb_sb = [persist.tile([128, T], F32, tag=f"xb{i}", name=f"xb_sb{i}") for i in range(2)]
        xbb = [persist.tile([128, T], BF16, tag=f"xbb{i}", name=f"xbb{i}") for i in range(2)]
        qk_sb = [persist.tile([128, T], BF16, tag=f"qk{i}", name=f"qk_sb{i}") for i in range(2)]  # [0]=q both heads, [1]=k both heads
        vt2 = persist.tile([128, NJP * 2 * 2 * 80], F8E4, tag="vt2")

        w_wqk = small.tile([128, 2, 256], BF16, tag="wqk")
        w_wv = small.tile([128, 2, 128], BF16, tag="wv")
        w2qk = small.tile([128, 2, 256], BF16, tag="w2qk")
        w2v = small.tile([128, 2, 128], BF16, tag="w2v")
        w_gagg = small.tile([128, 128], F32R, tag="gagg")
        w_pw = [small.tile([64, 256], F32R, tag=f"pw{i}", name=f"w_pw{i}")
                for i in range(2)]
        bvg2 = small.tile([64, 2], BF16, tag="bvg2")
        w_pwb = small.tile([64, 2, 256], BF16, tag="pwb")
        pwbv = small.tile([128, 2], F32, tag="pwbv")
        b_qk = small.tile([128, 2], F32, tag="bqk")
        bq2 = small.tile([128, 2], F32, tag="bq2")
        b_gnw = small.tile([128, 2], F32, tag="gnw")
        b_gnb = small.tile([128, 2], F32, tag="gnb")
        t_eps = small.tile([128, 1], F32, tag="eps")
        bi_ct = small.tile([128, 2], F32, tag="bict")

        vt2v = vt2[:].rearrange("p (c i h s) -> p c i h s", c=NJP, i=2, h=2,
                                s=80)

        # ================= S1: load x + GroupNorm stats + weight fold ======
        with tc.tile_pool(name="gn_ps", bufs=1, space="PSUM") as gn_ps, \
             tc.tile_pool(name="gn_tmp", bufs=4) as gn_tmp:
            for ct in range(2):
                for sub in range(8):
                    eng = (nc.sync, nc.gpsimd, nc.scalar, nc.gpsimd)[sub % 4]
                    eng.dma_start(xb_sb[ct][:, sub * 512:(sub + 1) * 512],
                                  d_xb[ct, :, sub * 512:(sub + 1) * 512])
            # weights + constants after the latency-critical xb load;
            # the many-descriptor vt2-ones DMA goes to the idle ACT queue
            nc.gpsimd.dma_start(
                vt2v[:, :, :, :, 64:65],
                d_cones8[:, :].rearrange("p (c i h u) -> p c i h u",
                                         c=NJP, i=2, h=2, u=1))
            nc.scalar.dma_start(w_gagg[:], d_gagg)
            nc.sync.dma_start(w_wqk[:], d_wqk.rearrange("k c o -> c k o"))
            nc.sync.dma_start(w_wv[:], d_wv.rearrange("k c o -> c k o"))
            for i in range(2):
                nc.sync.dma_start(w_pw[i][:], d_pw[i])
            nc.sync.dma_start(b_qk[:], d_bqk)
            nc.sync.dma_start(b_gnw[:], d_gnw)
            nc.sync.dma_start(b_gnb[:], d_gnb)
            nc.vector.memset(t_eps[:], EPS / 4)
            stats2 = []
            for ct in range(2):
                xt = xb_sb[ct]
                # cast raw x -> bf16 (per chunk, overlaps the DMA stream);
                # ACT is otherwise idle here
                stats = gn_tmp.tile([128, 8, 6], F32, tag=f"stats{ct}")
                stats2.append(stats)
                xv = xt[:].rearrange("p (n f) -> p n f", f=512)
                for sub in range(8):
                    nc.scalar.copy(xbb[ct][:, sub * 512:(sub + 1) * 512],
                                   xt[:, sub * 512:(sub + 1) * 512])
                    # PE warm-up: a throwaway matmul per arriving x chunk
                    # keeps the HAM clock gate at K=8/8 through the load
                    # phase so S2/S3 start at 2.4 GHz (output never read)
                    ps_w = gn_ps.tile([128, 512], F32, tag="warm")
                    nc.tensor.matmul(
                        ps_w[:], xbb[ct][:, sub * 512:sub * 512 + 128],
                        xbb[ct][:, sub * 512:(sub + 1) * 512],
                        start=True, stop=True)
                    nc.vector.bn_stats(stats[:, sub, :], xv[:, sub, :])
            for ct in range(2):
                mv = gn_tmp.tile([128, 2], F32, tag="mv")
                nc.vector.bn_aggr(mv[:], stats2[ct][:])
                # stats_in = [mean, var + mean^2] (f32r for the agg matmul)
                sin = gn_tmp.tile([128, 2], F32R, tag="sin")
                msq = gn_tmp.tile([128, 1], F32, tag="msq")
                nc.vector.tensor_mul(msq[:], mv[:, 0:1], mv[:, 0:1])
                nc.vector.tensor_copy(sin[:, 0:1], mv[:, 0:1])
                nc.vector.tensor_add(sin[:, 1:2], mv[:, 1:2], msq[:])
                ps_g = gn_ps.tile([128, 2], F32, tag="gps")
                nc.tensor.matmul(ps_g[:], w_gagg[:], sin[:], start=True, stop=True)
                g_sb = gn_tmp.tile([128, 2], F32, tag="gsb")
                nc.vector.tensor_copy(g_sb[:], ps_g[:])
                gm2 = gn_tmp.tile([128, 1], F32, tag="gm2")
                nc.vector.tensor_mul(gm2[:], g_sb[:, 0:1], g_sb[:, 0:1])
                gvar = gn_tmp.tile([128, 1], F32, tag="gvar")
                nc.vector.tensor_sub(gvar[:], g_sb[:, 1:2], gm2[:])
                srt = gn_tmp.tile([128, 1], F32, tag="srt")
                nc.scalar.activation(srt[:], gvar[:],
                                     mybir.ActivationFunctionType.Sqrt,
                                     bias=t_eps[:], scale=1.0)
                rstd = gn_tmp.tile([128, 1], F32, tag="rstd")
                nc.vector.reciprocal(rstd[:], srt[:])
                # GroupNorm folded into weights: normed = x*sc + bi with
                # sc = rstd*w, bi = b - gm*rstd*w (per input channel), so
                # w' = w*sc (per-partition scale) and the q/k biases pick up
                # w^T bi (computed below with tiny N=1 matmuls)
                sc = gn_tmp.tile([128, 1], F32, tag="sc")
                nc.vector.tensor_mul(sc[:], rstd[:], b_gnw[:, ct:ct + 1])
                bi = gn_tmp.tile([128, 1], F32, tag="bi")
                nc.vector.tensor_mul(bi[:], g_sb[:, 0:1], sc[:])
                nc.vector.tensor_sub(bi_ct[:, ct:ct + 1], b_gnb[:, ct:ct + 1], bi[:])
                nc.vector.tensor_scalar(
                    out=w2qk[:, ct, :], in0=w_wqk[:, ct, :],
                    scalar1=sc[:], scalar2=None,
                    op0=mybir.AluOpType.mult)
                nc.vector.tensor_scalar(
                    out=w2v[:, ct, :], in0=w_wv[:, ct, :],
                    scalar1=sc[:], scalar2=None,
                    op0=mybir.AluOpType.mult)
            # bias fold for q/k: bq2[:, ot] = sum_kt wqk[:, kt, ot]^T bi_kt + bqk
            bi_bf = gn_tmp.tile([128, 2], BF16, tag="bibf")
            nc.vector.tensor_copy(bi_bf[:], bi_ct[:])
            ps_b = gn_ps.tile([128, 2], F32, tag="bps")
            for ot in range(2):
                for kt in range(2):
                    nc.tensor.matmul(
                        ps_b[:, ot:ot + 1],
                        w_wqk[:, kt, ot * 128:(ot + 1) * 128],
                        bi_bf[:, kt:kt + 1],
                        start=(kt == 0), stop=(kt == 1))
            nc.vector.tensor_add(bq2[:], ps_b[:], b_qk[:])
            # v-side GN bias: bv_gn = sum_kt wv[:,kt]^T bi_kt (per v-channel);
            # softmax rows sum to 1 so it shifts h per channel -> apply
            # pw^T bv_gn as a per-partition bias on the proj output
            ps_v = gn_ps.tile([128, 1], F32, tag="vps")
            for kt in range(2):
                nc.tensor.matmul(ps_v[:], w_wv[:, kt, :], bi_bf[:, kt:kt + 1],
                                 start=(kt == 0), stop=(kt == 1))
            bvg = gn_tmp.tile([128, 1], BF16, tag="bvg")
            nc.vector.tensor_copy(bvg[:], ps_v[:])
            # fold the two heads' halves onto partitions 0-63 (DMA crossbar)
            nc.gpsimd.dma_start(bvg2[:, 0:1], bvg[0:64, :])
            nc.gpsimd.dma_start(bvg2[:, 1:2], bvg[64:128, :])
            for h in range(2):
                nc.vector.tensor_copy(w_pwb[:, h, :], w_pw[h][:])
            ps_pb = gn_ps.tile([128, 2], F32, tag="pbps")
            for ot in range(2):
                for h in range(2):
                    nc.tensor.matmul(ps_pb[:, ot:ot + 1],
                                     w_pwb[:, h, ot * 128:(ot + 1) * 128],
                                     bvg2[:, h:h + 1],
                                     start=(h == 0), stop=(h == 1))
            nc.vector.tensor_copy(pwbv[:], ps_pb[:])

        # ================= S3: attention + proj + out =================
        # Per s-tile j the two heads' QK matmuls are K=64 and sit in
        # disjoint PE row groups (partitions 0-63 / 64-127), so they run
        # CONCURRENTLY (tile_position auto-derived from base partitions).
        # exp is split across engines by j: ACT_J of 32 on ACT (true exp),
        # the rest on DVE via the Schraudolph int16 bit-trick -- the two
        # engines run in parallel, halving the softmax bottleneck.
        stage_pool = ctx.enter_context(tc.tile_pool(name="stage", bufs=4))
        den_pool = ctx.enter_context(tc.tile_pool(name="den", bufs=2))
        bcr_pool2 = ctx.enter_context(tc.tile_pool(name="bcr2", bufs=2))
        with tc.tile_pool(name="qk_ps", bufs=3, space="PSUM") as qk_ps, \
             tc.tile_pool(name="av_ps", bufs=2, space="PSUM") as av_ps, \
             tc.tile_pool(name="exp_sb", bufs=8) as exp_pool, \
             tc.tile_pool(name="osb", bufs=4) as osb_pool:
            def emit_qkv(ot, chk, on_dve=False):
                # one 512-col chunk of the q (ot=0) / k (ot=1) projection
                # (borrows a qs ring slot; PSUM is fully subscribed)
                ps2 = qk_ps.tile([128, 2, TC], F32, tag="qk")
                ps = ps2[:, 0, :]
                for kt in range(2):
                    nc.tensor.matmul(
                        ps[:], w2qk[:, kt, ot * 128:(ot + 1) * 128],
                        xbb[kt][:, chk * 512:(chk + 1) * 512],
                        start=(kt == 0), stop=(kt == 1))
                if on_dve:
                    nc.vector.tensor_scalar(
                        out=qk_sb[ot][:, chk * 512:(chk + 1) * 512], in0=ps[:],
                        scalar1=bq2[:, ot:ot + 1], scalar2=None,
                        op0=mybir.AluOpType.add)
                else:
                    nc.scalar.activation(
                        qk_sb[ot][:, chk * 512:(chk + 1) * 512], ps[:],
                        mybir.ActivationFunctionType.Identity,
                        bias=bq2[:, ot:ot + 1], scale=1.0)

            def emit_vt(ck2):
                # vT for s-tile pair ck2: out[s, c] = x[:, s].T @ Wv'[:, c]
                # (v bias is folded into the host-side output constant)
                pvt2 = qk_ps.tile([128, 2, TC], F32, tag="qk")
                pvt = pvt2[:, 0, 0:256]
                for sub in range(2):
                    chk = ck2 * 2 + sub
                    for kt in range(2):
                        nc.tensor.matmul(
                            pvt[:, sub * 128:(sub + 1) * 128],
                            xbb[kt][:, chk * 128:(chk + 1) * 128],
                            w2v[:, kt, :], start=(kt == 0), stop=(kt == 1))
                nc.scalar.copy(
                    vt2v[:, ck2, :, :, 0:64],
                    pvt[:].rearrange("p (i h s) -> p i h s", i=2, h=2))

            def emit_stage_h(tci, avs_p, sts_out, h):
                # avs -> SBUF stage copy + denominator reciprocal chain
                if True:
                    st = stage_pool.tile([65, TC], F32R, tag="stage",
                                         name=f"st{tci}_{h}")
                    nc.vector.tensor_copy(st[:], avs_p[h][:])
                    sts_out.append(st)
                    den0 = den_pool.tile([1, TC], F32, tag="den0",
                                         name=f"dn{tci}_{h}")
                    nc.gpsimd.dma_start(den0[:], st[64:65, :])
                    denR = den_pool.tile([1, TC], F32, tag="denR",
                                         name=f"dr{tci}_{h}")
                    nc.vector.reciprocal_approx_fast(denR[:], den0[:])
                    bcr2 = bcr_pool2.tile([64, TC], F32, tag="bcr2",
                                          name=f"bc{tci}_{h}")
                    nc.gpsimd.partition_broadcast(bcr2[:], denR[:], channels=64)
                    nc.vector.tensor_mul(st[0:64, :], st[0:64, :], bcr2[:])

            def emit_proj(tci, sts_p, ot):
                # proj + output for one t-chunk half (residual + static biases
                # on host; the runtime GN v-bias enters via the pwbv ACT bias)
                ps2 = qk_ps.tile([128, 2, TC], F32, tag="qk")
                ps = ps2[:, 0, :]
                for h in range(2):
                    nc.tensor.matmul(
                        ps[:], w_pw[h][:, ot * 128:(ot + 1) * 128],
                        sts_p[h][0:64, :],
                        start=(h == 0), stop=(h == 1))
                osb = osb_pool.tile([128, TC], F32, tag="osb")
                nc.scalar.activation(
                    osb[:], ps[:],
                    mybir.ActivationFunctionType.Identity,
                    bias=pwbv[:, ot:ot + 1], scale=1.0)
                (nc.sync if ot == 0 else nc.scalar).dma_start(
                    d_out[ot, :, tci * TC:(tci + 1) * TC], osb[:])

            prev = None   # (tci, avs) of the previous t-chunk
            emit_qkv(0, 0)
            emit_qkv(1, 0)
            emit_vt(0)
            for tci in range(NTC):
                # software pipeline: AV lags QK by two j so its exp wait
                # is pre-satisfied; the previous chunk's stage/normalize/proj
                # chain is interleaved into this chunk's j loop so the PE
                # never waits on it (a tci-boundary stall re-throttles HAM).
                # avs is allocated AFTER the previous chunk's stage copies so
                # the 2-deep PSUM ring orders the reuse correctly.
                avs = None
                pend = []
                sts_p = []
                for j in range(NST):
                    qs = qk_ps.tile([128, 2, TC], F32, tag="qk",
                                    name=f"qs{tci}_{j}")
                    for h in range(2):
                        nc.tensor.matmul(
                            qs[:, h, :],
                            qk_sb[1][h * 64:(h + 1) * 64, j * 128:(j + 1) * 128],
                            qk_sb[0][h * 64:(h + 1) * 64, tci * TC:(tci + 1) * TC],
                            start=True, stop=True)
                    if j == 0:
                        if prev is not None:
                            emit_stage_h(prev[0], prev[1], sts_p, 0)
                            emit_stage_h(prev[0], prev[1], sts_p, 1)
                        avs = [av_ps.tile([65, TC], F32, tag="av",
                                          name=f"av{tci}_{hh}")
                               for hh in range(2)]
                    elif prev is not None:
                        if j == 16:
                            emit_proj(prev[0], sts_p, 0)
                        elif j == 24:
                            emit_proj(prev[0], sts_p, 1)
                        elif j == 28 and tci < NTC - 1:
                            emit_qkv(0, tci + 1)
                    if tci == 0:
                        # interleave the rest of the k / vT production into
                        # the first chunk's loop (one s-tile group ahead of
                        # its consumer)
                        if j % 2 == 1 and (j + 1) // 2 < NJP:
                            emit_vt((j + 1) // 2)
                        if j % 4 == 3 and (j + 1) // 4 < 8:
                            emit_qkv(1, (j + 1) // 4)
                        if j == 28:
                            emit_qkv(0, 1)
                    # one exp instruction per j covering both heads (the two
                    # halves of the qs pair tile are adjacent PSUM banks) --
                    # amortizes the per-instruction engine startup; engines
                    # alternate by j parity
                    if j % 2 == 0:
                        ejq = exp_pool.tile([128, 2, 2, TC], F8E5, tag="exp",
                                            name=f"e{tci}_{j // 2}")
                    if j % 2 == 0 or j == NST - 1:
                        nc.scalar.activation(ejq[:, j % 2, :, :], qs[:],
                                             mybir.ActivationFunctionType.Exp,
                                             scale=SCALE2)
                    else:
                        nc.vector.tensor_scalar(
                            out=ejq[:, j % 2, :, :].bitcast(I8),
                            in0=qs[:],
                            scalar1=SCH_A8, scalar2=SCH_B8,
                            op0=mybir.AluOpType.mult,
                            op1=mybir.AluOpType.add)
                    if j % 2 == 1:
                        pend.append((j // 2, ejq))
                    while len(pend) > 3:
                        pjp, pe = pend.pop(0)
                        for ph in range(2):
                            nc.tensor.matmul(
                                avs[ph][:], vt2v[:, pjp, :, ph, 0:65],
                                pe[:, :, ph, :],
                                perf_mode=mybir.MatmulPerfMode.DoubleRow,
                                start=(pjp == 0), stop=(pjp == NJP - 1),
                                skip_group_check=True)
                for pjp, pe in pend:
                    for ph in range(2):
                        nc.tensor.matmul(
                            avs[ph][:], vt2v[:, pjp, :, ph, 0:65],
                            pe[:, :, ph, :],
                            perf_mode=mybir.MatmulPerfMode.DoubleRow,
                            start=(pjp == 0), stop=(pjp == NJP - 1),
                            skip_group_check=True)
                prev = (tci, avs)
            sts_p = []
            emit_stage_h(prev[0], prev[1], sts_p, 0)
            emit_stage_h(prev[0], prev[1], sts_p, 1)
            for ot in range(2):
                emit_proj(prev[0], sts_p, ot)


_NC_CACHE = None


def build_nc():
    global _NC_CACHE
    if _NC_CACHE is not None:
        return _NC_CACHE
    nc = bacc.Bacc("TRN2", target_bir_lowering=False, debug=False,
                   num_devices=N_CORES)
    with tile.TileContext(nc) as t:
        _emit(t)
    nc.compile()
    _NC_CACHE = nc
    return nc


def make_core_inputs(inputs, core):
    x = np.ascontiguousarray(np.asarray(inputs["x"], np.float32))
    norm_w = np.asarray(inputs["norm_w"], np.float32)
    norm_b = np.asarray(inputs["norm_b"], np.float32)
    qkv_w = np.asarray(inputs["qkv_w"], np.float32)
    qkv_b = np.asarray(inputs["qkv_b"], np.float32)
    proj_w = np.asarray(inputs["proj_w"], np.float32)
    b, p = core // 2, core % 2
    ha, hb = 2 * p, 2 * p + 1
    x2 = x.reshape(B, C, T)

    def rows(h, part):
        base = 192 * h + 64 * part
        return slice(base, base + 64)

    xb = np.ascontiguousarray(x2[b].reshape(2, 128, T))
    # o-tile 0 = [q_ha, q_hb], o-tile 1 = [k_ha, k_hb] (per-head slices of the
    # q / k SBUF tiles then share a base partition, which matmul requires)
    wqk_rows = np.concatenate([qkv_w[rows(ha, 0)], qkv_w[rows(hb, 0)],
                               qkv_w[rows(ha, 1)], qkv_w[rows(hb, 1)]], axis=0)
    wqk = np.ascontiguousarray(wqk_rows.T.reshape(2, 128, 256)).astype(ml_dtypes.bfloat16)
    bqk = np.ascontiguousarray(
        np.concatenate([qkv_b[rows(ha, 0)], qkv_b[rows(hb, 0)],
                        qkv_b[rows(ha, 1)], qkv_b[rows(hb, 1)]]).reshape(2, 128).T)
    wv_rows = np.concatenate([qkv_w[rows(ha, 2)], qkv_w[rows(hb, 2)]], axis=0)
    wv = np.ascontiguousarray(wv_rows.T.reshape(2, 128, 128)).astype(ml_dtypes.bfloat16)
    gnw = np.ascontiguousarray(norm_w.reshape(2, 128).T)
    gnb = np.ascontiguousarray(norm_b.reshape(2, 128).T)
    gagg = np.kron(np.eye(8, dtype=np.float32),
                   np.ones((16, 16), np.float32) / 16.0)
    pw = np.ascontiguousarray(
        proj_w[:, 128 * p:128 * p + 128].T.reshape(2, 64, 256))
    cones8 = np.ones((128, 64), ml_dtypes.float8_e4m3)
    return dict(xb=xb, wqk=wqk, bqk=bqk, wv=wv, gnw=gnw, gnb=gnb,
                gagg=gagg, pw=pw, cones8=cones8)


def _ensure_axon_devices():
    """The SPMD run needs the 8 axon-tunneled NeuronCores visible to jax.
    If a caller pinned jax to cpu (e.g. to run the reference), try to undo."""
    import jax
    try:
        if len(jax.devices("axon")) >= N_CORES:
            return
    except Exception:
        pass
    try:
        os.environ.pop("JAX_PLATFORMS", None)
        jax.config.update("jax_platforms", None)
        jax.extend.backend.clear_backends()
    except Exception:
        pass


def kernel(**inputs):
    try:
        import jax
        if not any(d.platform == "axon" for d in jax.devices()):
            _ensure_axon_devices()
    except Exception:
        _ensure_axon_devices()
    nc = build_nc()
    in_maps = [make_core_inputs(inputs, core) for core in range(N_CORES)]
    res = None
    last_err = None
    for attempt in range(4):
        try:
            res = run_bass_kernel_spmd(nc, in_maps, list(range(N_CORES)))
            break
        except Exception as e:  # transient NRT_EXEC_UNIT_UNRECOVERABLE etc.
            last_err = e
            import time as _time
            _time.sleep(2.0)
    if res is None:
        raise last_err
    x = np.asarray(inputs["x"], np.float32)
    qkv_b = np.asarray(inputs["qkv_b"], np.float32)
    proj_w = np.asarray(inputs["proj_w"], np.float32)
    proj_b = np.asarray(inputs["proj_b"], np.float32)
    x2 = x.reshape(B, C, T)
    # constant output bias: proj_b + proj_w @ v_bias (v bias never applied
    # on device; softmax rows sum to 1 so it contributes a constant)
    vb = np.empty((C,), np.float32)
    for c in range(C):
        vb[c] = qkv_b[192 * (c // 64) + 128 + (c % 64)]
    bias_all = proj_b + proj_w @ vb
    out = np.empty((B, C, T), np.float32)
    for b in range(B):
        out[b] = (res.results[2 * b]["out"].reshape(C, T)
                  + res.results[2 * b + 1]["out"].reshape(C, T)
                  + x2[b] + bias_all[:, None])
    return out.reshape(B, C, HH, WW)
